# revision 15
# baseline (speedup 1.0000x reference)
"""Trainium2 Bass kernel for nn_Block_16544214024520 (dense_cnn).

Data-parallel over batch: 16 samples -> 2 per NeuronCore x 8 cores.
All parameters replicated. Per-sample layout: channels on partitions
(256 = 2 chunks of 128), pixels (64x64 = 4096) on the free dim.

Reference pipeline (per sample):
  gn(32) -> 1x1 conv(256->256)+silu -> gn(16) -> 3x3 grouped conv
  (g=4, 256->512)+silu -> gn(2) -> window-mean(8x8) -> radix amax ->
  1x1 g-conv(256->64)+silu -> gn(8) -> 1x1 g-conv(64->512) ->
  softmax over radix(2) -> gated combine -> channel matmul(256->256)
  -> gn(32) -> +residual
"""

import os
import sys

for _p in ("/opt/trn_rl_repo", "/opt/pypackages"):
    if _p not in sys.path:
        sys.path.append(_p)

import numpy as np

import concourse.bass as bass  # noqa: F401
import concourse.mybir as mybir
import concourse.tile as tile
from concourse import bacc
from concourse.masks import make_identity

F32 = mybir.dt.float32
F32R = mybir.dt.float32r
AF = mybir.ActivationFunctionType
ALU = mybir.AluOpType
AX = mybir.AxisListType

NCORES = 8
BPC = 2          # samples per core
C = 256          # channels
H = W = 64
NPIX = H * W     # 4096
PADW = W + 2     # 66
Hn = Wn = 8      # window grid
WS = 8           # window size
EPS = 1e-5
NT = 8           # n-tiles of 512 pixels (8 rows of 64)


# ---------------------------------------------------------------- host prep

def _host_consts():
    """Constant matrices shared by all cores (built once)."""
    c = {}
    # GN over 256 channels, 32 groups of 8 (GN1/GN2/GN5)
    gm1 = np.zeros((2, 128, 32), np.float32)
    rep1 = np.zeros((2, 128, 128), np.float32)
    for ch in range(2):
        for k in range(128):
            g = (128 * ch + k) // 8
            gm1[ch, k, g] = 1.0 / 8.0
        for m in range(128):
            rep1[ch, (128 * ch + m) // 8 % 128, m] = 1.0
    c["gm1"] = gm1
    c["rep1"] = rep1
    # GN2: 16 groups of 16 over 256 channels
    gm2 = np.zeros((2, 128, 16), np.float32)
    rep2 = np.zeros((2, 128, 128), np.float32)
    for ch in range(2):
        for k in range(128):
            gm2[ch, k, (128 * ch + k) // 16] = 1.0 / 16.0
        for m in range(128):
            rep2[ch, (128 * ch + m) // 16, m] = 1.0
    c["gm2"] = gm2
    c["rep2"] = rep2
    # GN3 over 512 channels, 2 groups of 256 (chunks 0,1 -> g0; 2,3 -> g1)
    g3 = np.zeros((4, 128, 2), np.float32)
    r3 = np.zeros((4, 128, 128), np.float32)
    for mc in range(4):
        g3[mc, :, mc // 2] = 1.0 / 256.0
        r3[mc, mc // 2, :] = 1.0
    c["g3"] = g3
    c["r3"] = r3
    # GN4 over 64 channels, 8 groups of 8
    g4 = np.zeros((128, 8), np.float32)
    for k in range(64):
        g4[k, k // 8] = 1.0 / 8.0
    r4 = np.zeros((128, 64), np.float32)
    for m in range(64):
        r4[m // 8, m] = 1.0
    c["g4"] = g4
    c["r4"] = r4
    return c


def _host_weights(w0, b0, w1, b1, w2, b2, w3, b3, weight):
    """Rearrange torch-layout conv weights into matmul lhsT tensors."""
    d = {}
    # conv0: out[o,p] = sum_i w0[o,i] x[i,p]  -> lhsT[i,o]
    d["w0T"] = np.ascontiguousarray(w0[:, :, 0, 0].T).astype(np.float32)  # [256,256]
    d["b0c"] = np.ascontiguousarray(b0.reshape(C, 1)).astype(np.float32)
    # conv1: grouped 3x3, groups=4 (in 64 -> out 128 each), natural order.
    # w1t[tap, kc] is a [128, 256] block: rows = in-ch of groups (2kc, 2kc+1),
    # col block 0 (0:128) = out chunk 2kc (uses rows 0:64),
    # col block 1 (128:256) = out chunk 2kc+1 (uses rows 64:128).
    w1t = np.zeros((9, 2, 128, 256), np.float32)
    for tap in range(9):
        dy, dx = tap // 3, tap % 3
        for kc in range(2):
            for blk in range(2):
                g = 2 * kc + blk
                # out channels g*128 + j, in-ch local r in 0..63
                w1t[tap, kc, blk * 64:(blk + 1) * 64, blk * 128:(blk + 1) * 128] = \
                    w1[g * 128:(g + 1) * 128, :, dy, dx].T
    d["w1t"] = w1t
    d["b1c"] = np.ascontiguousarray(b1.reshape(2 * C, 1)).astype(np.float32)
    # conv2: groups=2 (in 128 -> out 32); fold the 1/64 window mean here.
    w2t = np.zeros((2, 128, 32), np.float32)
    for g in range(2):
        w2t[g] = w2[g * 32:(g + 1) * 32, :, 0, 0].T / float(WS * WS)
    d["w2t"] = w2t
    d["b2c"] = np.ascontiguousarray(b2.reshape(64, 1)).astype(np.float32)
    # conv3: groups=2 (in 32 -> out 256); K padded to 128 with zero rows.
    w3t = np.zeros((4, 128, 128), np.float32)
    for g in range(4):
        src = w3[g * 128:(g + 1) * 128, :, 0, 0]      # [128, 32]
        r0 = 0 if g < 2 else 32
        w3t[g, r0:r0 + 32, :] = src.T
    d["w3t"] = w3t
    # final einsum: out[c,p] = sum_C weight[C,c] z[C,p], z[C] = zint[2C]+zint[2C+1]
    # fold the radix pair-sum by duplicating rows: wdup[c512, c] = weight[c512//2, c]
    wdup = np.repeat(weight.astype(np.float32), 2, axis=0)   # [512, 256]
    d["wdupT"] = np.ascontiguousarray(wdup)
    return d


# ---------------------------------------------------------------- builder

def build_nc(sim_safe: bool = False):
    nc = bacc.Bacc("TRN2", target_bir_lowering=False, debug=False,
                   num_devices=NCORES)

    def din(name, shape, dt=F32):
        return nc.dram_tensor(name, list(shape), dt, kind="ExternalInput").ap()

    hs = din("hs", (BPC, C, H, W), F32R)
    w0T = din("w0T", (C, C))
    b0c = din("b0c", (C, 1))
    w1t = din("w1t", (9, 2, 128, 256), F32R)
    b1c = din("b1c", (2 * C, 1))
    w2t = din("w2t", (2, 128, 32))
    b2c = din("b2c", (64, 1))
    w3t = din("w3t", (4, 128, 128))
    wdupT = din("wdupT", (2 * C, C), F32R)
    gm1 = din("gm1", (2, 128, 32))
    rep1 = din("rep1", (2, 128, 128))
    gm2 = din("gm2", (2, 128, 16))
    rep2 = din("rep2", (2, 128, 128))
    g3 = din("g3", (4, 128, 2))
    r3 = din("r3", (4, 128, 128))
    g4 = din("g4", (128, 8))
    r4 = din("r4", (128, 64))

    out_d = nc.dram_tensor("out", [BPC, C, H, W], F32, kind="ExternalOutput").ap()

    with tile.TileContext(nc) as tc:
        with tc.tile_pool(name="consts", bufs=1) as cst, \
             tc.tile_pool(name="big", bufs=1) as big, \
             tc.tile_pool(name="small", bufs=2) as sm, \
             tc.tile_pool(name="psum", bufs=2, space="PSUM") as psp:

            # ---- load constants / weights (resident) ----
            w0_t = [cst.tile([128, 256], F32, name=f"w0_{c}") for c in range(2)]
            for c in range(2):
                nc.sync.dma_start(out=w0_t[c], in_=w0T[c * 128:(c + 1) * 128, :])
            b0_t = [cst.tile([128, 1], F32, name=f"b0_{m}") for m in range(2)]
            for m in range(2):
                nc.sync.dma_start(out=b0_t[m], in_=b0c[m * 128:(m + 1) * 128, :])
            w1_t = [[cst.tile([128, 256], F32R, name=f"w1_{t}_{k}")
                     for k in range(2)] for t in range(9)]
            for t in range(9):
                for k in range(2):
                    nc.sync.dma_start(out=w1_t[t][k], in_=w1t[t, k])
            b1_t = [cst.tile([128, 1], F32, name=f"b1_{g}") for g in range(4)]
            for g in range(4):
                nc.sync.dma_start(out=b1_t[g], in_=b1c[g * 128:(g + 1) * 128, :])
            w2_t = [cst.tile([128, 32], F32, name=f"w2_{g}") for g in range(2)]
            for g in range(2):
                nc.sync.dma_start(out=w2_t[g], in_=w2t[g])
            b2_t = cst.tile([128, 1], F32, name="b2")
            nc.vector.memset(b2_t, 0.0)
            nc.sync.dma_start(out=b2_t[0:64, :], in_=b2c)
            w3_t = [cst.tile([128, 128], F32, name=f"w3_{g}") for g in range(4)]
            for g in range(4):
                nc.sync.dma_start(out=w3_t[g], in_=w3t[g])
            wd_t = [cst.tile([128, 256], F32R, name=f"wd_{k}") for k in range(4)]
            for k in range(4):
                nc.sync.dma_start(out=wd_t[k], in_=wdupT[k * 128:(k + 1) * 128, :])
            gm1_t = [cst.tile([128, 32], F32, name=f"gm1_{c}") for c in range(2)]
            rep1_t = [cst.tile([128, 128], F32, name=f"rep1_{c}") for c in range(2)]
            gm2_t = [cst.tile([128, 16], F32, name=f"gm2_{c}") for c in range(2)]
            rep2_t = [cst.tile([128, 128], F32, name=f"rep2_{c}") for c in range(2)]
            for c in range(2):
                nc.sync.dma_start(out=gm1_t[c], in_=gm1[c])
                nc.sync.dma_start(out=rep1_t[c], in_=rep1[c])
                nc.sync.dma_start(out=gm2_t[c], in_=gm2[c])
                nc.sync.dma_start(out=rep2_t[c], in_=rep2[c])
            g3_t = [cst.tile([128, 2], F32, name=f"g3_{g}") for g in range(4)]
            r3_t = [cst.tile([128, 128], F32, name=f"r3_{g}") for g in range(4)]
            for g in range(4):
                nc.sync.dma_start(out=g3_t[g], in_=g3[g])
                nc.sync.dma_start(out=r3_t[g], in_=r3[g])
            g4_t = cst.tile([128, 8], F32, name="g4")
            nc.sync.dma_start(out=g4_t, in_=g4)
            r4_t = cst.tile([128, 64], F32, name="r4")
            nc.sync.dma_start(out=r4_t, in_=r4)
            ident = cst.tile([128, 128], F32, name="ident")
            make_identity(nc, ident)
            eps_t = cst.tile([128, 1], F32, name="eps")
            nc.vector.memset(eps_t, EPS)

            # ------------------------------------------------ helpers
            def silu_evac(out_ap, psum_ap, bias_ap, tag):
                """out = silu(psum + bias); fused on HW, 2-op in CoreSim."""
                if not sim_safe:
                    nc.scalar.activation(out=out_ap, in_=psum_ap, func=AF.Silu,
                                         bias=bias_ap, scale=1.0)
                else:
                    sgf = sm.tile([128, 512], F32, tag="sg", bufs=1,
                                  name=f"sg_{tag}", uniquify=True)
                    pp = psum_ap.partition_size()
                    ff = psum_ap.free_size()
                    sgt = sgf[0:pp, 0:ff]
                    nc.scalar.activation(out=sgt, in_=psum_ap, func=AF.Sigmoid,
                                         bias=bias_ap, scale=1.0)
                    nc.vector.scalar_tensor_tensor(
                        out=out_ap, in0=psum_ap, scalar=bias_ap, in1=sgt,
                        op0=ALU.add, op1=ALU.mult)

            def gn_scale_bias(mvs, gmat_list, rmat_list, ngroups, tag):
                """Per-channel (scale, bias) tiles for a group norm.

                mvs: list of [128, 2] SBUF tiles of per-channel (mean, var),
                valid on the partition ranges covered by gmat rows.
                Returns list of [128, 2] tiles (col0 = rstd, col1 = -mean*rstd)
                replicated back to channels, one per input chunk.
                """
                nchunk = len(mvs)
                # per-channel [mean, E[x^2]]
                rstats = []
                for ci, mv in enumerate(mvs):
                    r = sm.tile([128, 2], F32, tag=f"r_{tag}", bufs=2 * nchunk)
                    nc.vector.tensor_copy(out=r[:, 0:1], in_=mv[:, 0:1])
                    nc.vector.scalar_tensor_tensor(
                        out=r[:, 1:2], in0=mv[:, 0:1], scalar=mv[:, 0:1],
                        in1=mv[:, 1:2], op0=ALU.mult, op1=ALU.add)
                    rstats.append(r)
                pg = psp.tile([128, 2], F32, tag="gn_ps", bufs=2)
                for ci in range(nchunk):
                    nc.tensor.matmul(pg[0:ngroups, :], gmat_list[ci], rstats[ci],
                                     start=(ci == 0), stop=(ci == nchunk - 1))
                gt = sm.tile([128, 2], F32, tag=f"gt_{tag}", bufs=2)
                nc.vector.memset(gt, 0.0)
                nc.scalar.copy(out=gt[0:ngroups, :], in_=pg[0:ngroups, :])
                # -var = mean^2 - E[x^2]
                negv = sm.tile([128, 1], F32, tag=f"nv_{tag}", bufs=2)
                nc.vector.scalar_tensor_tensor(
                    out=negv[0:ngroups], in0=gt[0:ngroups, 0:1],
                    scalar=gt[0:ngroups, 0:1], in1=gt[0:ngroups, 1:2],
                    op0=ALU.mult, op1=ALU.subtract)
                sd = sm.tile([128, 1], F32, tag=f"sd_{tag}", bufs=2)
                nc.scalar.activation(out=sd[0:ngroups], in_=negv[0:ngroups],
                                     func=AF.Sqrt, bias=eps_t[0:ngroups],
                                     scale=-1.0)
                rstd = sm.tile([128, 1], F32, tag=f"rs_{tag}", bufs=2)
                nc.vector.reciprocal(out=rstd[0:ngroups], in_=sd[0:ngroups])
                stg = sm.tile([128, 2], F32, tag=f"st_{tag}", bufs=2)
                nc.vector.memset(stg, 0.0)
                nc.vector.tensor_copy(out=stg[0:ngroups, 0:1], in_=rstd[0:ngroups])
                nc.vector.tensor_scalar(
                    out=stg[0:ngroups, 1:2], in0=gt[0:ngroups, 0:1],
                    scalar1=rstd[0:ngroups], scalar2=-1.0,
                    op0=ALU.mult, op1=ALU.mult)
                scs = []
                for ci, rmat in enumerate(rmat_list):
                    mm = rmat.shape[-1]
                    pr = psp.tile([128, 2], F32, tag="gn_ps", bufs=2)
                    nc.tensor.matmul(pr[0:mm, :], rmat, stg,
                                     start=True, stop=True)
                    sc = sm.tile([128, 2], F32, tag=f"sc_{tag}", bufs=2 * nchunk)
                    nc.scalar.copy(out=sc[0:mm, :], in_=pr[0:mm, :])
                    scs.append(sc)
                return scs

            def chan_stats(src_list, tag, nsub=NT):
                """bn_stats/bn_aggr per chunk -> [128,2] (mean, var) tiles."""
                mvs = []
                for ci, src in enumerate(src_list):
                    if src.dtype == F32R:
                        src = src.bitcast(F32)
                    free = src.free_size()
                    sub = free // 512 if free >= 512 else 1
                    bst = sm.tile([128, max(sub, 1), 6], F32,
                                  tag=f"bst_{tag}", bufs=2)
                    if sub > 1:
                        srcv = src.rearrange("p (a b) -> p a b", a=sub)
                        for si in range(sub):
                            nc.vector.bn_stats(out=bst[:, si, :],
                                               in_=srcv[:, si, :])
                    else:
                        nc.vector.bn_stats(out=bst,
                                           in_=src.unsqueeze(1))
                    mv = sm.tile([128, 2], F32, tag=f"mv_{tag}",
                                 bufs=2 * len(src_list))
                    nc.vector.bn_aggr(out=mv, in_=bst)
                    mvs.append(mv)
                return mvs

            # ------------------------------------------------ per-sample body
            for b in range(BPC):
                hsv = hs[b].rearrange("c h w -> c (h w)")   # [256, 4096]

                # load input chunks
                xw = [big.tile([128, NPIX], F32R, tag="xw", bufs=3, padded_shape=[128, PADW * PADW], name=f"xw{b}_{i}")
                      for i in range(2)]
                for c in range(2):
                    nc.sync.dma_start(out=xw[c],
                                      in_=hsv[c * 128:(c + 1) * 128, :])

                # ---- GN1 stats; fold normalization into conv0 weights ----
                mv1 = chan_stats(xw, "gn1")
                sc1 = gn_scale_bias(mv1, gm1_t, rep1_t, 32, "gn1")
                w0s = [sm.tile([128, 256], F32R, tag="w0s", bufs=2, name=f"w0s{b}_{i}")
                       for i in range(2)]
                for c in range(2):
                    nc.vector.tensor_scalar_mul(out=w0s[c], in0=w0_t[c],
                                                scalar1=sc1[c][:, 0:1])
                # bias correction: b0' = b0 + sum_i w0s[i, o] * t_i
                b0p = [sm.tile([128, 1], F32, tag="b0p", bufs=4, name=f"b0p{b}_{i}")
                       for i in range(2)]
                for m in range(2):
                    pb = psp.tile([128, 1], F32, tag="gn_ps", bufs=2)
                    for kc in range(2):
                        nc.tensor.matmul(
                            pb,
                            w0s[kc][:, m * 128:(m + 1) * 128].bitcast(F32),
                            sc1[kc][:, 1:2],
                            start=(kc == 0), stop=(kc == 1))
                    nc.scalar.activation(out=b0p[m], in_=pb, func=AF.Identity,
                                         bias=b0_t[m], scale=1.0)

                # ---- conv0 (1x1, 256->256) + silu -> y0 ----
                y0 = [big.tile([128, NPIX], F32, tag="y0", bufs=2, name=f"y0{b}_{i}")
                      for i in range(2)]
                for n in range(NT):
                    nsl = bass.ts(n, 512)
                    for m in range(2):
                        pt = psp.tile([128, 512], F32, tag="acc", bufs=4)
                        for kc in range(2):
                            nc.tensor.matmul(
                                pt,
                                w0s[kc][:, m * 128:(m + 1) * 128],
                                xw[kc][:, nsl],
                                start=(kc == 0), stop=(kc == 1))
                        silu_evac(y0[m][:, nsl], pt, b0p[m], "c0")

                # ---- GN2 -> write normalized into padded buffer xp ----
                mv2 = chan_stats(y0, "gn2")
                sc2 = gn_scale_bias(mv2, gm2_t, rep2_t, 16, "gn2")
                xp = [big.tile([128, PADW, PADW], F32R, tag="xw", bufs=3, name=f"xp{b}_{i}")
                      for i in range(2)]
                for c in range(2):
                    # zero the 1-pixel border
                    xpf = xp[c].bitcast(F32)
                    nc.gpsimd.memset(xpf[:, 0:1, :], 0.0)
                    nc.gpsimd.memset(xpf[:, PADW - 1:PADW, :], 0.0)
                    nc.gpsimd.memset(xpf[:, 1:PADW - 1, 0:1], 0.0)
                    nc.gpsimd.memset(xpf[:, 1:PADW - 1, PADW - 1:PADW], 0.0)
                    nc.gpsimd.tensor_scalar(
                        out=xp[c][:, 1:H + 1, 1:W + 1],
                        in0=y0[c].rearrange("p (h w) -> p h w", h=H),
                        scalar1=sc2[c][:, 0:1], scalar2=sc2[c][:, 1:2],
                        op0=ALU.mult, op1=ALU.add)

                # ---- conv1 (3x3 grouped, 256->512) + silu -> y1 ----
                # group-pure K=64 matmuls; groups 2kc / 2kc+1 sit at partition
                # bases 0 / 64 so the PE row-tiles them concurrently.
                y1 = [big.tile([128, NPIX], F32R, tag="y1", bufs=4, name=f"y1{b}_{i}")
                      for i in range(4)]
                for n in range(NT):
                    r0 = n * WS
                    for kc in range(2):
                        pts = [psp.tile([128, 512], F32, tag="acc", bufs=4,
                                        name=f"pc1_{b}_{n}_{kc}_{i}")
                               for i in range(2)]
                        for tap in range(9):
                            dy, dx = tap // 3 - 1, tap % 3 - 1
                            for blk in range(2):
                                p0 = blk * 64
                                rhs = xp[kc][p0:p0 + 64,
                                             r0 + 1 + dy:r0 + 9 + dy,
                                             1 + dx:W + 1 + dx]
                                lhsT = w1_t[tap][kc][p0:p0 + 64,
                                                     blk * 128:(blk + 1) * 128]
                                nc.tensor.matmul(
                                    pts[blk], lhsT, rhs,
                                    start=(tap == 0), stop=(tap == 8))
                        for blk in range(2):
                            g = 2 * kc + blk
                            silu_evac(y1[g][:, bass.ts(n, 512)], pts[blk],
                                      b1_t[g], "c1")

                # ---- GN3 stats -> normalize y1 in place (-> yn) ----
                mv3 = chan_stats(y1, "gn3")
                sc3 = gn_scale_bias(mv3, g3_t, r3_t, 2, "gn3")
                for g in range(4):
                    eng = nc.gpsimd if g % 2 == 0 else nc.vector
                    eng.tensor_scalar(
                        out=y1[g], in0=y1[g].bitcast(F32),
                        scalar1=sc3[g][:, 0:1], scalar2=sc3[g][:, 1:2],
                        op0=ALU.mult, op1=ALU.add)

                # ---- window mean + radix amax (transposed) ----
                pooled = [sm.tile([128, Hn, Wn], F32, tag="pooled", bufs=4, name=f"pooled{b}_{i}")
                          for i in range(4)]
                for g in range(4):
                    yv = y1[g].bitcast(F32).rearrange(
                        "p (a w2) -> p a w2", w2=WS)        # a = (hn h2 wn)
                    pa = sm.tile([128, Hn * WS * Wn], F32, tag="scratch512", bufs=2,
                                 name=f"poolA{b}_{g}", uniquify=True)
                    nc.vector.tensor_reduce(
                        out=pa, in_=yv, axis=AX.X, op=ALU.add)
                    pav = pa.rearrange("p (hn h2 wn) -> p hn wn h2",
                                       hn=Hn, h2=WS)
                    nc.vector.tensor_reduce(
                        out=pooled[g], in_=pav, axis=AX.X, op=ALU.add)
                pooledT = [sm.tile([64, 128], F32, tag="pooledT", bufs=4, name=f"pooledT{b}_{i}")
                           for i in range(4)]
                for g in range(4):
                    ptp = psp.tile([64, 128], F32, tag="tp", bufs=2)
                    nc.tensor.transpose(
                        ptp, pooled[g].rearrange("p a b -> p (a b)"), ident)
                    nc.scalar.copy(out=pooledT[g], in_=ptp)
                amT = sm.tile([64, 256], F32, tag="amT", bufs=1)
                for g in range(4):
                    pv = pooledT[g].rearrange("p (a b) -> p a b", b=2)
                    nc.vector.tensor_tensor(
                        out=amT[:, g * 64:(g + 1) * 64],
                        in0=pv[:, :, 0], in1=pv[:, :, 1], op=ALU.max)
                am = [sm.tile([128, 64], F32, tag="am", bufs=4, name=f"am{b}_{i}")
                      for i in range(2)]
                for c in range(2):
                    pta = psp.tile([128, 64], F32, tag="tp", bufs=2)
                    nc.tensor.transpose(pta, amT[:, c * 128:(c + 1) * 128],
                                        ident[0:64, 0:64])
                    nc.scalar.copy(out=am[c], in_=pta)

                # ---- conv2 (1x1 g=2, 256->64) + silu ----
                p2 = psp.tile([128, 64], F32, tag="tp", bufs=2)
                for g in range(2):
                    nc.tensor.matmul(p2[g * 32:(g + 1) * 32, :], w2_t[g], am[g],
                                     start=True, stop=True)
                a2 = sm.tile([128, 64], F32, tag="a2", bufs=2)
                nc.vector.memset(a2, 0.0)
                silu_evac(a2[0:64, :], p2[0:64, :], b2_t[0:64], "c2")

                # ---- GN4 -> a2n ----
                mv4pad = sm.tile([128, 2], F32, tag="mv4", bufs=2)
                nc.vector.memset(mv4pad, 0.0)
                bst4 = sm.tile([128, 1, 6], F32, tag="bst4", bufs=2)
                nc.vector.bn_stats(out=bst4[0:64], in_=a2[0:64].unsqueeze(1))
                nc.vector.bn_aggr(out=mv4pad[0:64], in_=bst4[0:64])
                sc4 = gn_scale_bias([mv4pad], [g4_t], [r4_t], 8, "gn4")[0]
                a2n = sm.tile([128, 64], F32, tag="a2n", bufs=2)
                nc.vector.memset(a2n, 0.0)
                nc.vector.tensor_scalar(
                    out=a2n[0:64], in0=a2[0:64],
                    scalar1=sc4[0:64, 0:1], scalar2=sc4[0:64, 1:2],
                    op0=ALU.mult, op1=ALU.add)

                # ---- conv3 (1x1 g=2, 64->512), b3 = 0 ----
                a3T = sm.tile([64, 512], F32, tag="a3T", bufs=1)
                for g in range(4):
                    p3 = psp.tile([128, 64], F32, tag="tp", bufs=2)
                    nc.tensor.matmul(p3, w3_t[g], a2n, start=True, stop=True)
                    a3 = sm.tile([128, 64], F32, tag="a3", bufs=4)
                    nc.scalar.copy(out=a3, in_=p3)
                    p3t = psp.tile([64, 128], F32, tag="tp", bufs=2)
                    nc.tensor.transpose(p3t, a3, ident)
                    nc.scalar.copy(out=a3T[:, g * 128:(g + 1) * 128], in_=p3t)

                # ---- softmax over radix == sigmoid of pair difference ----
                a3v = a3T.rearrange("p (a b) -> p a b", b=2)
                dT = sm.tile([64, 256], F32, tag="dT", bufs=1)
                nc.vector.tensor_tensor(out=dT, in0=a3v[:, :, 0],
                                        in1=a3v[:, :, 1], op=ALU.subtract)
                sT = sm.tile([64, 512], F32, tag="sT", bufs=1)
                sTv = sT.rearrange("p (a b) -> p a b", b=2)
                nc.scalar.activation(out=sTv[:, :, 0], in_=dT,
                                     func=AF.Sigmoid, scale=1.0)
                nc.scalar.activation(out=sTv[:, :, 1], in_=dT,
                                     func=AF.Sigmoid, scale=-1.0)
                sint = [sm.tile([128, 64], F32, tag="sint", bufs=8, name=f"sint{b}_{i}")
                        for i in range(4)]
                for g in range(4):
                    pst = psp.tile([128, 64], F32, tag="tp", bufs=2)
                    nc.tensor.transpose(pst, sT[:, g * 128:(g + 1) * 128],
                                        ident[0:64, 0:64])
                    nc.scalar.copy(out=sint[g], in_=pst)

                # ---- gated combine: zint = yn * window-bcast(sint) ----
                for g in range(4):
                    # stage 1: [p, hn, wn] -> [p, hn, wn, w2] (one window row)
                    srow = sm.tile([128, Hn, Wn, WS], F32, tag="scratch512", bufs=2,
                                   name=f"srow{b}_{g}", uniquify=True)
                    sbv = sint[g].rearrange("p (hn wn) -> p hn wn", hn=Hn)
                    nc.vector.tensor_copy(
                        out=srow,
                        in_=sbv.unsqueeze(3).broadcast_to([128, Hn, Wn, WS]))
                    # stage 2: broadcast over h2 (the 8 rows of each window)
                    yv = y1[g].rearrange("p (hn h2 x) -> p hn h2 x",
                                         hn=Hn, h2=WS)
                    sbig = srow.rearrange("p hn wn w2 -> p hn (wn w2)")
                    sbig = sbig.unsqueeze(2).broadcast_to([128, Hn, WS, Wn * WS])
                    nc.vector.tensor_tensor(out=yv, in0=yv.bitcast(F32),
                                            in1=sbig, op=ALU.mult)

                # ---- final channel matmul (K=512 dup -> 256) ----
                ot = [big.tile([128, NPIX], F32, tag="y0", bufs=2, name=f"ot{b}_{i}")
                      for i in range(2)]
                for n in range(NT):
                    nsl = bass.ts(n, 512)
                    for m in range(2):
                        pt = psp.tile([128, 512], F32, tag="acc", bufs=4)
                        for kc in range(4):
                            nc.tensor.matmul(
                                pt,
                                wd_t[kc][:, m * 128:(m + 1) * 128],
                                y1[kc][:, nsl],
                                start=(kc == 0), stop=(kc == 3))
                        nc.scalar.copy(out=ot[m][:, nsl], in_=pt)

                # ---- GN5 + residual ----
                mv5 = chan_stats(ot, "gn5")
                sc5 = gn_scale_bias(mv5, gm1_t, rep1_t, 32, "gn5")
                xr = [big.tile([128, NPIX], F32R, tag="xw", bufs=3, padded_shape=[128, PADW * PADW], name=f"xr{b}_{i}")
                      for i in range(2)]
                for c in range(2):
                    nc.sync.dma_start(out=xr[c],
                                      in_=hsv[c * 128:(c + 1) * 128, :])
                ov = out_d[b].rearrange("c h w -> c (h w)")
                for c in range(2):
                    nc.scalar.activation(out=ot[c], in_=ot[c], func=AF.Identity,
                                         bias=sc5[c][:, 1:2],
                                         scale=sc5[c][:, 0:1])
                    nc.vector.tensor_tensor(out=ot[c], in0=ot[c],
                                            in1=xr[c].bitcast(F32), op=ALU.add)
                    nc.sync.dma_start(out=ov[c * 128:(c + 1) * 128, :],
                                      in_=ot[c])

    nc.compile()
    return nc


# ---------------------------------------------------------------- entry

_CACHE = {}


def _get_nc(sim_safe=False):
    key = bool(sim_safe)
    if key not in _CACHE:
        _CACHE[key] = build_nc(sim_safe=key)
    return _CACHE[key]


def make_in_maps(inputs):
    hs_full = np.ascontiguousarray(inputs["hidden_state"], dtype=np.float32)
    wd = _host_weights(
        np.asarray(inputs["w0"], np.float32), np.asarray(inputs["b0"], np.float32),
        np.asarray(inputs["w1"], np.float32), np.asarray(inputs["b1"], np.float32),
        np.asarray(inputs["w2"], np.float32), np.asarray(inputs["b2"], np.float32),
        np.asarray(inputs["w3"], np.float32), np.asarray(inputs["b3"], np.float32),
        np.asarray(inputs["weight"], np.float32))
    cm = _host_consts()
    shared = {
        "w0T": wd["w0T"], "b0c": wd["b0c"], "w1t": wd["w1t"], "b1c": wd["b1c"],
        "w2t": wd["w2t"], "b2c": wd["b2c"], "w3t": wd["w3t"],
        "wdupT": wd["wdupT"],
        "gm1": cm["gm1"], "rep1": cm["rep1"], "gm2": cm["gm2"],
        "rep2": cm["rep2"], "g3": cm["g3"], "r3": cm["r3"],
        "g4": cm["g4"], "r4": cm["r4"],
    }
    in_maps = []
    for i in range(NCORES):
        m = dict(shared)
        m["hs"] = np.ascontiguousarray(hs_full[i * BPC:(i + 1) * BPC])
        in_maps.append(m)
    return in_maps


def kernel(**inputs):
    from concourse import bass_utils
    nc = _get_nc(sim_safe=False)
    in_maps = make_in_maps(inputs)
    res = bass_utils.run_bass_kernel_spmd(nc, in_maps,
                                          core_ids=list(range(NCORES)))
    out = np.concatenate([res.results[i]["out"] for i in range(NCORES)], axis=0)
    return out.astype(np.float32)
